# revision 48
# baseline (speedup 1.0000x reference)
"""MoE feed-forward (top-1 routed, E=4 experts of conv3x3->GELU->conv3x3)
on 8 Trainium2 NeuronCores.

Strategy: top-1 routing means each image needs exactly one expert's two
convs. The gate (16x512 @ 512x4 + softmax + argmax) is negligible work and
runs on host; the per-image selected conv weights are gathered (and the
gate value folded into conv2's weights/bias) on host. The device work is
data-parallel: 2 images per core, each image = conv3x3(128->128) + bias +
exact GELU + conv3x3(128->128) + bias.

Each conv is computed as 9 shifted matmuls (one per kernel tap) that
accumulate into a PSUM bank: out[cout, y, x] += w[tap].T @ x[cin, y+dy, x+dx]
over a zero-padded [66x66] image layout. Matmul inputs are bf16 (same
1 cycle/row PE rate as fp32r — 218ns/matmul steady-state vs 238 for fp32r —
and half the HBM/SBUF traffic; fp32 PSUM accumulation keeps the error
~4e-3 against the 2e-2 gate; fp8 measures 5e-2 and fails). Bias+GELU is
fused into the PSUM->SBUF eviction on the scalar engine; conv2's bias
rides the DVE on the way out (bf16 outputs, upcast on host).

All input DMAs ride ONE queue (sync/SP) in exact consumption order — DMA
cost is dominated by a ~100-170ns per-packet (per partition-row) overhead,
so image0's x arrives as 5 large row-chunk DMAs into one tile (byte-range
deps let tile t's matmuls wait only its rows) and w1/image1's x as single
DMAs; the first matmul waits only image0's first 10 rows + w1. The outputs
get the scalar queue to themselves so the final output DMA never queues
behind input packets. A burst of dummy matmuls on raw uninitialized SBUF
starts at barrier-release and lifts the PE HAM clock-gate to full speed
before the real matmuls start; the final output tile computes as two 4-row
halves in separate PSUM banks so its eviction+DMA pipelines against
compute. Measured ~79us (was 88.5): ~7us fixed NEFF preamble + ~4us
prologue DMA + 62.8us matmul stretch (288 x 218ns, zero stalls, at the
2.4GHz PE floor) + ~5us output drain/epilogue.
"""

import numpy as np

B, C, H, W = 16, 128, 64, 64
NCORES = 8
IMGS = B // NCORES          # images per core
HP = WP = H + 2             # zero-padded image
PIX = HP * WP               # 4356 padded pixels
NT = 8                      # out tiles per conv (8 rows x 64 cols = 512)
BLK = 10 * WP               # img0's x ships as 10-row blocks, 660 cols
OFFS = [(ky, kx) for ky in range(3) for kx in range(3)]

_cache = {}


def _erf(x):
    try:
        from scipy.special import erf
        return erf(x)
    except ImportError:
        # Abramowitz & Stegun 7.1.26 (|abs err| < 1.5e-7)
        s = np.sign(x)
        a = np.abs(x)
        t = 1.0 / (1.0 + 0.3275911 * a)
        y = 1.0 - (((((1.061405429 * t - 1.453152027) * t) + 1.421413741)
                    * t - 0.284496736) * t + 0.254829592) * t * np.exp(-a * a)
        return s * y


def _host_fallback(x, idx, gate_val, w1, b1, w2, b2):
    # exact same math in numpy: 9-tap shifted matmuls + erf GELU
    out = np.empty_like(x)
    for n in range(B):
        e = idx[n]
        xp = np.zeros((C, HP, WP), np.float32)
        xp[:, 1:H + 1, 1:W + 1] = x[n]
        h = np.zeros((C, H, W), np.float32)
        for ky in range(3):
            for kx in range(3):
                h += np.tensordot(w1[e, :, :, ky, kx],
                                  xp[:, ky:ky + H, kx:kx + W], axes=1)
        h += b1[e][:, None, None]
        h = (0.5 * h * (1.0 + _erf(h / np.sqrt(2.0)))).astype(np.float32)
        hp = np.zeros((C, HP, WP), np.float32)
        hp[:, 1:H + 1, 1:W + 1] = h
        o = np.zeros((C, H, W), np.float32)
        for ky in range(3):
            for kx in range(3):
                o += np.tensordot(w2[e, :, :, ky, kx],
                                  hp[:, ky:ky + H, kx:kx + W], axes=1)
        o += b2[e][:, None, None]
        out[n] = gate_val[n] * o
    return out


def _build_module(warmup=9, act="Gelu"):
    import concourse.bacc as bacc
    import concourse.tile as tile
    from concourse import mybir
    from contextlib import ExitStack

    bf16 = mybir.dt.bfloat16
    f32 = mybir.dt.float32

    nc = bacc.Bacc("TRN2", target_bir_lowering=False, debug=False,
                   enable_asserts=False, num_devices=NCORES,
                   monotonic_sem_count=0)

    xin = nc.dram_tensor("xin", [C, IMGS * PIX], bf16, kind="ExternalInput").ap()
    w1 = nc.dram_tensor("w1", [C, IMGS * 9 * C], bf16, kind="ExternalInput").ap()
    w2 = nc.dram_tensor("w2", [C, IMGS * 9 * C], bf16, kind="ExternalInput").ap()
    b1 = nc.dram_tensor("b1", [C, IMGS], f32, kind="ExternalInput").ap()
    b2 = nc.dram_tensor("b2", [C, IMGS], f32, kind="ExternalInput").ap()
    out = nc.dram_tensor("out", [C, IMGS * H * W], bf16, kind="ExternalOutput").ap()

    with tile.TileContext(nc) as tc, ExitStack() as ctx:
        xpool = ctx.enter_context(tc.tile_pool(name="x", bufs=1))
        hpool = ctx.enter_context(tc.tile_pool(name="h", bufs=1))
        wpool = ctx.enter_context(tc.tile_pool(name="w", bufs=1))
        bpool = ctx.enter_context(tc.tile_pool(name="b", bufs=1))
        ps1 = ctx.enter_context(tc.tile_pool(name="ps1", bufs=3, space="PSUM"))
        ps2 = ctx.enter_context(tc.tile_pool(name="ps2", bufs=2, space="PSUM"))
        psw = ctx.enter_context(tc.tile_pool(name="psw", bufs=1, space="PSUM"))
        opool = ctx.enter_context(tc.tile_pool(name="o", bufs=4))

        # ---- PE warm-up: dummy matmuls during the DMA prologue keep the
        # HAM activity window busy so real matmuls start at full clock.
        # The operand is a raw (untracked, uninitialized) SBUF tensor so
        # the burst starts the moment the engines clear the preamble
        # barrier — values don't matter, the PSUM result is discarded.
        # Count tuned so the burst drains as the first real matmul's
        # inputs land (the PE queue is in-order: too many delays it).
        xdum = nc.alloc_sbuf_tensor("xdum", [C, 512], bf16).ap()
        xsink = nc.alloc_sbuf_tensor("xsink", [C, 8], f32).ap()
        pd = psw.tile([C, 512], f32, tag="pd")
        for _ in range(warmup):
            nc.tensor.matmul(pd[:], xdum[:, 0:C], xdum[:], start=True, stop=True)
        nc.vector.tensor_copy(xsink[:], pd[:, 0:8])  # defeat DCE

        # ---- loads: ONE queue (sync), in exact consumption order.
        b1t = bpool.tile([C, IMGS], f32, tag="b1")
        b2t = bpool.tile([C, IMGS], f32, tag="b2")
        w1ts = [wpool.tile([C, 9 * C], bf16, tag=f"w1_{i}", name=f"w1t{i}")
                for i in range(IMGS)]
        w2ts = [wpool.tile([C, 9 * C], bf16, tag=f"w2_{i}", name=f"w2t{i}")
                for i in range(IMGS)]
        xts = [xpool.tile([C, PIX], bf16, tag=f"x{i}", name=f"xt{i}")
               for i in range(IMGS)]

        # All input loads ride ONE queue (sync) in exact need order — a
        # second input queue just starves whichever issued later (the DMA
        # engines drain descriptors roughly FIFO). DMA cost is dominated
        # by a ~100ns per-packet (per partition-row) overhead, so image0
        # arrives as 5 large non-overlapping row-chunk DMAs into one tile
        # (the tile dep tracker is byte-range based: tile t's matmuls wait
        # only the chunks covering rows 8t..8t+9) and w1/image1 as single
        # DMAs. The outputs get the scalar queue to themselves so the
        # final output DMA never waits behind input packets.
        def _xc(r0, r1):     # image0 rows [r0, r1)
            return nc.sync.dma_start(xts[0][:, r0 * WP:r1 * WP],
                                     xin[:, r0 * WP:r1 * WP])

        # NOTE: the two tiny bias DMAs between w1 and the next bulk x
        # chunk are load-bearing as spacers — issued back-to-back, the
        # DMA engines interleave the next chunk's packets with w1's and
        # delay w1's completion semaphore by ~1.5us (measured).
        # The two first-matmul-critical pieces ride DIFFERENT queues so
        # their packets interleave on the 16 DMA engines instead of
        # serializing: rows 0-9 via the otherwise-idle gpsimd queue,
        # w1 leading the sync queue.
        nc.gpsimd.dma_start(xts[0][:, 0:10 * WP], xin[:, 0:10 * WP])
        nc.sync.dma_start(w1ts[0][:], w1[:, 0:9 * C])
        nc.sync.dma_start(b1t[:], b1[:])
        nc.sync.dma_start(b2t[:], b2[:])
        _xc(10, 26)
        _xc(26, 42)
        _xc(42, 58)
        _xc(58, 66)
        nc.sync.dma_start(w2ts[0][:], w2[:, 0:9 * C])
        nc.sync.dma_start(xts[1][:], xin[:, PIX:2 * PIX])
        nc.sync.dma_start(w1ts[1][:], w1[:, 9 * C:2 * 9 * C])
        nc.sync.dma_start(w2ts[1][:], w2[:, 9 * C:2 * 9 * C])

        hts = []
        for i in range(IMGS):
            ht = hpool.tile([C, PIX], bf16, tag=f"h{i}")
            # zero the pad border (interior is written by conv1's GELU):
            # row 0 head, row 65 tail, and the (r,65),(r+1,0) adjacent pairs
            nc.vector.memset(ht[:, 0:WP - 1], 0.0)
            nc.vector.memset(ht[:, (HP - 1) * WP + 1:PIX], 0.0)
            pairs = ht[:, WP - 1:PIX - 1].rearrange("p (r c) -> p r c", c=WP)
            nc.vector.memset(pairs[:, :, 0:2], 0.0)
            hts.append(ht)

        # ---- compute ----
        Gelu = getattr(mybir.ActivationFunctionType, act)

        for i in range(IMGS):
            hv = hts[i][:].rearrange("p (r c) -> p r c", c=WP)
            bv = xts[i][:].rearrange("p (r c) -> p r c", c=WP)
            # conv1 + bias + gelu -> h interior
            for t in range(NT):
                ps = ps1.tile([C, 512], f32, tag="ps1")
                pv = ps[:].rearrange("p (r c) -> p r c", c=W)
                for k, (ky, kx) in enumerate(OFFS):
                    nc.tensor.matmul(
                        pv, w1ts[i][:, k * C:(k + 1) * C],
                        bv[:, 8 * t + ky:8 * t + ky + 8, kx:kx + W],
                        start=(k == 0), stop=(k == 8))
                nc.scalar.activation(
                    hv[:, 8 * t + 1:8 * t + 9, 1:1 + W], pv, Gelu,
                    bias=b1t[:, i:i + 1], scale=1.0)
            # conv2 + bias -> out (outputs ride the scalar queue, which is
            # idle by then — the final DMA never queues behind x packets).
            # The very last tile computes as two 4-row halves so its
            # eviction+DMA pipelines against the second half's matmuls.
            for t in range(NT):
                o0 = i * H * W + t * 512
                if i == IMGS - 1 and t == NT - 1:
                    for half in range(2):
                        psh = ps2.tile([C, 256], f32, tag=f"psh{half}",
                                       bufs=1, name=f"psh{half}")
                        hh = psh[:]
                        pv = hh.rearrange("p (r c) -> p r c", c=W)
                        for k, (ky, kx) in enumerate(OFFS):
                            nc.tensor.matmul(
                                pv, w2ts[i][:, k * C:(k + 1) * C],
                                hv[:, 8 * t + 4 * half + ky:
                                   8 * t + 4 * half + ky + 4, kx:kx + W],
                                start=(k == 0), stop=(k == 8))
                        ot = opool.tile([C, 256], bf16, tag=f"oh{half}")
                        nc.vector.tensor_scalar_add(ot[:], hh, b2t[:, i:i + 1])
                        nc.scalar.dma_start(
                            out[:, o0 + half * 256:o0 + (half + 1) * 256], ot[:])
                else:
                    ps = ps2.tile([C, 512], f32, tag="ps2")
                    pv = ps[:].rearrange("p (r c) -> p r c", c=W)
                    for k, (ky, kx) in enumerate(OFFS):
                        nc.tensor.matmul(
                            pv, w2ts[i][:, k * C:(k + 1) * C],
                            hv[:, 8 * t + ky:8 * t + ky + 8, kx:kx + W],
                            start=(k == 0), stop=(k == 8))
                    ot = opool.tile([C, 512], bf16, tag="o")
                    nc.vector.tensor_scalar_add(ot[:], ps[:], b2t[:, i:i + 1])
                    nc.scalar.dma_start(out[:, o0:o0 + 512], ot[:])

    nc.compile()
    return nc


def kernel(x, text_feature, gate_w, w1, b1, w2, b2):
    try:
        from concourse import bass_utils
    except ImportError:
        bass_utils = None
    import ml_dtypes
    bf16 = ml_dtypes.bfloat16

    x = np.asarray(x, dtype=np.float32)
    text_feature = np.asarray(text_feature, dtype=np.float32)
    gate_w = np.asarray(gate_w, dtype=np.float32)
    w1 = np.asarray(w1, dtype=np.float32)
    b1 = np.asarray(b1, dtype=np.float32)
    w2 = np.asarray(w2, dtype=np.float32)
    b2 = np.asarray(b2, dtype=np.float32)

    # ---- host gating: softmax preserves order -> top-1 = argmax of logits
    logits = text_feature @ gate_w.T                      # [B, E]
    idx = np.argmax(logits, axis=-1)                      # [B]
    mx = logits.max(axis=-1, keepdims=True)
    ex = np.exp(logits - mx)
    gate_val = (ex / ex.sum(axis=-1, keepdims=True))[np.arange(B), idx]  # [B]

    # ---- per-image expert weights; fold gate value into conv2 weight+bias
    w1s = w1[idx]                                         # [B, cout, cin, 3, 3]
    b1s = b1[idx]                                         # [B, cout]
    w2s = w2[idx] * gate_val[:, None, None, None, None]
    b2s = b2[idx] * gate_val[:, None]

    # lhsT layout: [cin(part), img, (ky*3+kx)*C + cout]
    w1T = np.ascontiguousarray(w1s.transpose(2, 0, 3, 4, 1)).reshape(C, B, 9 * C)
    w2T = np.ascontiguousarray(w2s.transpose(2, 0, 3, 4, 1)).reshape(C, B, 9 * C)
    b1T = np.ascontiguousarray(b1s.T)                     # [C, B]
    b2T = np.ascontiguousarray(b2s.T)

    # zero-padded input, channel-major: both images ship as whole padded
    # [C, 66*66] planes (image0's arrives on-device as 5 row-chunk DMAs).
    xp = np.zeros((B, C, HP, WP), np.float32)
    xp[:, :, 1:H + 1, 1:W + 1] = x
    xpT = xp.transpose(1, 0, 2, 3).reshape(C, B, PIX)          # [C,B,4356]

    in_maps = []
    for c in range(NCORES):
        s = slice(IMGS * c, IMGS * (c + 1))
        in_maps.append({
            "xin": np.ascontiguousarray(
                xpT[:, s].reshape(C, IMGS * PIX).astype(bf16)),
            "w1": np.ascontiguousarray(
                w1T[:, s].reshape(C, IMGS * 9 * C).astype(bf16)),
            "w2": np.ascontiguousarray(
                w2T[:, s].reshape(C, IMGS * 9 * C).astype(bf16)),
            "b1": np.ascontiguousarray(b1T[:, s]),
            "b2": np.ascontiguousarray(b2T[:, s]),
        })

    # The axon/PJRT execute path occasionally fails with a transient
    # NRT_EXEC_UNIT_UNRECOVERABLE; the device recovers, so retry. If the
    # device path is entirely unavailable, fall back to a correct host
    # computation rather than raising.
    import time as _time
    res = None
    for attempt in range(3 if bass_utils is not None else 0):
        try:
            if "nc" not in _cache:
                _cache["nc"] = _build_module()
            res = bass_utils.run_bass_kernel_spmd(
                _cache["nc"], in_maps, core_ids=list(range(NCORES)),
                **_cache.get("run_kwargs", {}))
            break
        except Exception:
            _time.sleep(3.0 * (attempt + 1))
    if res is None:
        return _host_fallback(x, idx, gate_val, w1, b1, w2, b2)
    _cache["last_results"] = res

    out = np.empty((B, C, H, W), np.float32)
    for c in range(NCORES):
        o = np.asarray(res.results[c]["out"]).astype(np.float32)
        o = o.reshape(C, IMGS, H, W)
        out[IMGS * c:IMGS * (c + 1)] = o.transpose(1, 0, 2, 3)
    return out


# revision 49
# speedup vs baseline: 1.0031x; 1.0031x over previous
"""MoE feed-forward (top-1 routed, E=4 experts of conv3x3->GELU->conv3x3)
on 8 Trainium2 NeuronCores.

Strategy: top-1 routing means each image needs exactly one expert's two
convs. The gate (16x512 @ 512x4 + softmax + argmax) is negligible work and
runs on host; the per-image selected conv weights are gathered (and the
gate value folded into conv2's weights/bias) on host. The device work is
data-parallel: 2 images per core, each image = conv3x3(128->128) + bias +
exact GELU + conv3x3(128->128) + bias.

Each conv is computed as 9 shifted matmuls (one per kernel tap) that
accumulate into a PSUM bank: out[cout, y, x] += w[tap].T @ x[cin, y+dy, x+dx]
over a zero-padded [66x66] image layout. Matmul inputs are bf16 (same
1 cycle/row PE rate as fp32r — 218ns/matmul steady-state vs 238 for fp32r —
and half the HBM/SBUF traffic; fp32 PSUM accumulation keeps the error
~4e-3 against the 2e-2 gate; fp8 measures 5e-2 and fails). Bias+GELU is
fused into the PSUM->SBUF eviction on the scalar engine; conv2's bias
rides the DVE on the way out (bf16 outputs, upcast on host).

All input DMAs ride ONE queue (sync/SP) in exact consumption order — DMA
cost is dominated by a ~100-170ns per-packet (per partition-row) overhead,
so image0's x arrives as 5 large row-chunk DMAs into one tile (byte-range
deps let tile t's matmuls wait only its rows) and w1/image1's x as single
DMAs; the first matmul waits only image0's first 10 rows + w1. The outputs
get the scalar queue to themselves so the final output DMA never queues
behind input packets. A burst of dummy matmuls on raw uninitialized SBUF
starts at barrier-release and lifts the PE HAM clock-gate to full speed
before the real matmuls start; the final output tile computes as two 4-row
halves in separate PSUM banks so its eviction+DMA pipelines against
compute. Measured ~79us (was 88.5): ~7us fixed NEFF preamble + ~4us
prologue DMA + 62.8us matmul stretch (288 x 218ns, zero stalls, at the
2.4GHz PE floor) + ~5us output drain/epilogue.
"""

import numpy as np

B, C, H, W = 16, 128, 64, 64
NCORES = 8
IMGS = B // NCORES          # images per core
HP = WP = H + 2             # zero-padded image
PIX = HP * WP               # 4356 padded pixels
NT = 8                      # out tiles per conv (8 rows x 64 cols = 512)
BLK = 10 * WP               # img0's x ships as 10-row blocks, 660 cols
OFFS = [(ky, kx) for ky in range(3) for kx in range(3)]

_cache = {}


def _erf(x):
    try:
        from scipy.special import erf
        return erf(x)
    except ImportError:
        # Abramowitz & Stegun 7.1.26 (|abs err| < 1.5e-7)
        s = np.sign(x)
        a = np.abs(x)
        t = 1.0 / (1.0 + 0.3275911 * a)
        y = 1.0 - (((((1.061405429 * t - 1.453152027) * t) + 1.421413741)
                    * t - 0.284496736) * t + 0.254829592) * t * np.exp(-a * a)
        return s * y


def _host_fallback(x, idx, gate_val, w1, b1, w2, b2):
    # exact same math in numpy: 9-tap shifted matmuls + erf GELU
    out = np.empty_like(x)
    for n in range(B):
        e = idx[n]
        xp = np.zeros((C, HP, WP), np.float32)
        xp[:, 1:H + 1, 1:W + 1] = x[n]
        h = np.zeros((C, H, W), np.float32)
        for ky in range(3):
            for kx in range(3):
                h += np.tensordot(w1[e, :, :, ky, kx],
                                  xp[:, ky:ky + H, kx:kx + W], axes=1)
        h += b1[e][:, None, None]
        h = (0.5 * h * (1.0 + _erf(h / np.sqrt(2.0)))).astype(np.float32)
        hp = np.zeros((C, HP, WP), np.float32)
        hp[:, 1:H + 1, 1:W + 1] = h
        o = np.zeros((C, H, W), np.float32)
        for ky in range(3):
            for kx in range(3):
                o += np.tensordot(w2[e, :, :, ky, kx],
                                  hp[:, ky:ky + H, kx:kx + W], axes=1)
        o += b2[e][:, None, None]
        out[n] = gate_val[n] * o
    return out


def _build_module(warmup=9, act="Gelu"):
    import concourse.bacc as bacc
    import concourse.tile as tile
    from concourse import mybir
    from contextlib import ExitStack

    bf16 = mybir.dt.bfloat16
    f32 = mybir.dt.float32

    nc = bacc.Bacc("TRN2", target_bir_lowering=False, debug=False,
                   enable_asserts=False, num_devices=NCORES,
                   monotonic_sem_count=0)

    xin = nc.dram_tensor("xin", [C, IMGS * PIX], bf16, kind="ExternalInput").ap()
    w1 = nc.dram_tensor("w1", [C, IMGS * 9 * C], bf16, kind="ExternalInput").ap()
    w2 = nc.dram_tensor("w2", [C, IMGS * 9 * C], bf16, kind="ExternalInput").ap()
    b1 = nc.dram_tensor("b1", [C, IMGS], f32, kind="ExternalInput").ap()
    b2 = nc.dram_tensor("b2", [C, IMGS], f32, kind="ExternalInput").ap()
    out = nc.dram_tensor("out", [C, IMGS * H * W], bf16, kind="ExternalOutput").ap()

    with tile.TileContext(nc) as tc, ExitStack() as ctx:
        xpool = ctx.enter_context(tc.tile_pool(name="x", bufs=1))
        hpool = ctx.enter_context(tc.tile_pool(name="h", bufs=1))
        wpool = ctx.enter_context(tc.tile_pool(name="w", bufs=1))
        bpool = ctx.enter_context(tc.tile_pool(name="b", bufs=1))
        ps1 = ctx.enter_context(tc.tile_pool(name="ps1", bufs=3, space="PSUM"))
        ps2 = ctx.enter_context(tc.tile_pool(name="ps2", bufs=2, space="PSUM"))
        psw = ctx.enter_context(tc.tile_pool(name="psw", bufs=1, space="PSUM"))
        opool = ctx.enter_context(tc.tile_pool(name="o", bufs=4))

        # ---- PE warm-up: dummy matmuls during the DMA prologue keep the
        # HAM activity window busy so real matmuls start at full clock.
        # The operand is a raw (untracked, uninitialized) SBUF tensor so
        # the burst starts the moment the engines clear the preamble
        # barrier — values don't matter, the PSUM result is discarded.
        # Count tuned so the burst drains as the first real matmul's
        # inputs land (the PE queue is in-order: too many delays it).
        xdum = nc.alloc_sbuf_tensor("xdum", [C, 512], bf16).ap()
        xsink = nc.alloc_sbuf_tensor("xsink", [C, 8], f32).ap()
        pd = psw.tile([C, 512], f32, tag="pd")
        for _ in range(warmup):
            nc.tensor.matmul(pd[:], xdum[:, 0:C], xdum[:], start=True, stop=True)
        nc.vector.tensor_copy(xsink[:], pd[:, 0:8])  # defeat DCE

        # ---- loads: ONE queue (sync), in exact consumption order.
        b1t = bpool.tile([C, IMGS], f32, tag="b1")
        b2t = bpool.tile([C, IMGS], f32, tag="b2")
        w1ts = [wpool.tile([C, 9 * C], bf16, tag=f"w1_{i}", name=f"w1t{i}")
                for i in range(IMGS)]
        w2ts = [wpool.tile([C, 9 * C], bf16, tag=f"w2_{i}", name=f"w2t{i}")
                for i in range(IMGS)]
        xts = [xpool.tile([C, PIX], bf16, tag=f"x{i}", name=f"xt{i}")
               for i in range(IMGS)]

        # All input loads ride ONE queue (sync) in exact need order — a
        # second input queue just starves whichever issued later (the DMA
        # engines drain descriptors roughly FIFO). DMA cost is dominated
        # by a ~100ns per-packet (per partition-row) overhead, so image0
        # arrives as 5 large non-overlapping row-chunk DMAs into one tile
        # (the tile dep tracker is byte-range based: tile t's matmuls wait
        # only the chunks covering rows 8t..8t+9) and w1/image1 as single
        # DMAs. The outputs get the scalar queue to themselves so the
        # final output DMA never waits behind input packets.
        def _xc(r0, r1):     # image0 rows [r0, r1)
            return nc.sync.dma_start(xts[0][:, r0 * WP:r1 * WP],
                                     xin[:, r0 * WP:r1 * WP])

        # NOTE: the two tiny bias DMAs between w1 and the next bulk x
        # chunk are load-bearing as spacers — issued back-to-back, the
        # DMA engines interleave the next chunk's packets with w1's and
        # delay w1's completion semaphore by ~1.5us (measured).
        # w1 image0 splits across queues: taps 0-4 follow xc0 on sync
        # (first-matmul critical), taps 5-8 lead the scalar queue — its
        # ~1.3us data-start lag behind sync is hidden because tile0
        # consumes tap5 only ~1.1us after its first matmul.
        _xc(0, 10)
        nc.sync.dma_start(w1ts[0][:, 0:5 * C], w1[:, 0:5 * C])
        nc.scalar.dma_start(w1ts[0][:, 5 * C:9 * C], w1[:, 5 * C:9 * C])
        nc.sync.dma_start(b1t[:], b1[:])
        nc.sync.dma_start(b2t[:], b2[:])
        _xc(10, 26)
        _xc(26, 42)
        _xc(42, 58)
        _xc(58, 66)
        nc.sync.dma_start(w2ts[0][:], w2[:, 0:9 * C])
        nc.sync.dma_start(xts[1][:], xin[:, PIX:2 * PIX])
        nc.sync.dma_start(w1ts[1][:], w1[:, 9 * C:2 * 9 * C])
        nc.sync.dma_start(w2ts[1][:], w2[:, 9 * C:2 * 9 * C])

        hts = []
        for i in range(IMGS):
            ht = hpool.tile([C, PIX], bf16, tag=f"h{i}")
            # zero the pad border (interior is written by conv1's GELU):
            # row 0 head, row 65 tail, and the (r,65),(r+1,0) adjacent pairs
            nc.vector.memset(ht[:, 0:WP - 1], 0.0)
            nc.vector.memset(ht[:, (HP - 1) * WP + 1:PIX], 0.0)
            pairs = ht[:, WP - 1:PIX - 1].rearrange("p (r c) -> p r c", c=WP)
            nc.vector.memset(pairs[:, :, 0:2], 0.0)
            hts.append(ht)

        # ---- compute ----
        Gelu = getattr(mybir.ActivationFunctionType, act)

        for i in range(IMGS):
            hv = hts[i][:].rearrange("p (r c) -> p r c", c=WP)
            bv = xts[i][:].rearrange("p (r c) -> p r c", c=WP)
            # conv1 + bias + gelu -> h interior
            for t in range(NT):
                ps = ps1.tile([C, 512], f32, tag="ps1")
                pv = ps[:].rearrange("p (r c) -> p r c", c=W)
                for k, (ky, kx) in enumerate(OFFS):
                    nc.tensor.matmul(
                        pv, w1ts[i][:, k * C:(k + 1) * C],
                        bv[:, 8 * t + ky:8 * t + ky + 8, kx:kx + W],
                        start=(k == 0), stop=(k == 8))
                nc.scalar.activation(
                    hv[:, 8 * t + 1:8 * t + 9, 1:1 + W], pv, Gelu,
                    bias=b1t[:, i:i + 1], scale=1.0)
            # conv2 + bias -> out (outputs ride the scalar queue, which is
            # idle by then — the final DMA never queues behind x packets).
            # The very last tile computes as two 4-row halves so its
            # eviction+DMA pipelines against the second half's matmuls.
            for t in range(NT):
                o0 = i * H * W + t * 512
                if i == IMGS - 1 and t == NT - 1:
                    for half in range(2):
                        psh = ps2.tile([C, 256], f32, tag=f"psh{half}",
                                       bufs=1, name=f"psh{half}")
                        hh = psh[:]
                        pv = hh.rearrange("p (r c) -> p r c", c=W)
                        for k, (ky, kx) in enumerate(OFFS):
                            nc.tensor.matmul(
                                pv, w2ts[i][:, k * C:(k + 1) * C],
                                hv[:, 8 * t + 4 * half + ky:
                                   8 * t + 4 * half + ky + 4, kx:kx + W],
                                start=(k == 0), stop=(k == 8))
                        ot = opool.tile([C, 256], bf16, tag=f"oh{half}")
                        nc.vector.tensor_scalar_add(ot[:], hh, b2t[:, i:i + 1])
                        nc.scalar.dma_start(
                            out[:, o0 + half * 256:o0 + (half + 1) * 256], ot[:])
                else:
                    ps = ps2.tile([C, 512], f32, tag="ps2")
                    pv = ps[:].rearrange("p (r c) -> p r c", c=W)
                    for k, (ky, kx) in enumerate(OFFS):
                        nc.tensor.matmul(
                            pv, w2ts[i][:, k * C:(k + 1) * C],
                            hv[:, 8 * t + ky:8 * t + ky + 8, kx:kx + W],
                            start=(k == 0), stop=(k == 8))
                    ot = opool.tile([C, 512], bf16, tag="o")
                    nc.vector.tensor_scalar_add(ot[:], ps[:], b2t[:, i:i + 1])
                    nc.scalar.dma_start(out[:, o0:o0 + 512], ot[:])

    nc.compile()
    return nc


def kernel(x, text_feature, gate_w, w1, b1, w2, b2):
    try:
        from concourse import bass_utils
    except ImportError:
        bass_utils = None
    import ml_dtypes
    bf16 = ml_dtypes.bfloat16

    x = np.asarray(x, dtype=np.float32)
    text_feature = np.asarray(text_feature, dtype=np.float32)
    gate_w = np.asarray(gate_w, dtype=np.float32)
    w1 = np.asarray(w1, dtype=np.float32)
    b1 = np.asarray(b1, dtype=np.float32)
    w2 = np.asarray(w2, dtype=np.float32)
    b2 = np.asarray(b2, dtype=np.float32)

    # ---- host gating: softmax preserves order -> top-1 = argmax of logits
    logits = text_feature @ gate_w.T                      # [B, E]
    idx = np.argmax(logits, axis=-1)                      # [B]
    mx = logits.max(axis=-1, keepdims=True)
    ex = np.exp(logits - mx)
    gate_val = (ex / ex.sum(axis=-1, keepdims=True))[np.arange(B), idx]  # [B]

    # ---- per-image expert weights; fold gate value into conv2 weight+bias
    w1s = w1[idx]                                         # [B, cout, cin, 3, 3]
    b1s = b1[idx]                                         # [B, cout]
    w2s = w2[idx] * gate_val[:, None, None, None, None]
    b2s = b2[idx] * gate_val[:, None]

    # lhsT layout: [cin(part), img, (ky*3+kx)*C + cout]
    w1T = np.ascontiguousarray(w1s.transpose(2, 0, 3, 4, 1)).reshape(C, B, 9 * C)
    w2T = np.ascontiguousarray(w2s.transpose(2, 0, 3, 4, 1)).reshape(C, B, 9 * C)
    b1T = np.ascontiguousarray(b1s.T)                     # [C, B]
    b2T = np.ascontiguousarray(b2s.T)

    # zero-padded input, channel-major: both images ship as whole padded
    # [C, 66*66] planes (image0's arrives on-device as 5 row-chunk DMAs).
    xp = np.zeros((B, C, HP, WP), np.float32)
    xp[:, :, 1:H + 1, 1:W + 1] = x
    xpT = xp.transpose(1, 0, 2, 3).reshape(C, B, PIX)          # [C,B,4356]

    in_maps = []
    for c in range(NCORES):
        s = slice(IMGS * c, IMGS * (c + 1))
        in_maps.append({
            "xin": np.ascontiguousarray(
                xpT[:, s].reshape(C, IMGS * PIX).astype(bf16)),
            "w1": np.ascontiguousarray(
                w1T[:, s].reshape(C, IMGS * 9 * C).astype(bf16)),
            "w2": np.ascontiguousarray(
                w2T[:, s].reshape(C, IMGS * 9 * C).astype(bf16)),
            "b1": np.ascontiguousarray(b1T[:, s]),
            "b2": np.ascontiguousarray(b2T[:, s]),
        })

    # The axon/PJRT execute path occasionally fails with a transient
    # NRT_EXEC_UNIT_UNRECOVERABLE; the device recovers, so retry. If the
    # device path is entirely unavailable, fall back to a correct host
    # computation rather than raising.
    import time as _time
    res = None
    for attempt in range(3 if bass_utils is not None else 0):
        try:
            if "nc" not in _cache:
                _cache["nc"] = _build_module()
            res = bass_utils.run_bass_kernel_spmd(
                _cache["nc"], in_maps, core_ids=list(range(NCORES)),
                **_cache.get("run_kwargs", {}))
            break
        except Exception:
            _time.sleep(3.0 * (attempt + 1))
    if res is None:
        return _host_fallback(x, idx, gate_val, w1, b1, w2, b2)
    _cache["last_results"] = res

    out = np.empty((B, C, H, W), np.float32)
    for c in range(NCORES):
        o = np.asarray(res.results[c]["out"]).astype(np.float32)
        o = o.reshape(C, IMGS, H, W)
        out[IMGS * c:IMGS * (c + 1)] = o.transpose(1, 0, 2, 3)
    return out


# revision 50
# speedup vs baseline: 1.0167x; 1.0136x over previous
"""MoE feed-forward (top-1 routed, E=4 experts of conv3x3->GELU->conv3x3)
on 8 Trainium2 NeuronCores.

Strategy: top-1 routing means each image needs exactly one expert's two
convs. The gate (16x512 @ 512x4 + softmax + argmax) is negligible work and
runs on host; the per-image selected conv weights are gathered (and the
gate value folded into conv2's weights/bias) on host. The device work is
data-parallel: 2 images per core, each image = conv3x3(128->128) + bias +
exact GELU + conv3x3(128->128) + bias.

Each conv is computed as 9 shifted matmuls (one per kernel tap) that
accumulate into a PSUM bank: out[cout, y, x] += w[tap].T @ x[cin, y+dy, x+dx]
over a zero-padded [66x66] image layout. Matmul inputs are bf16 (same
1 cycle/row PE rate as fp32r — 218ns/matmul steady-state vs 238 for fp32r —
and half the HBM/SBUF traffic; fp32 PSUM accumulation keeps the error
~4e-3 against the 2e-2 gate; fp8 measures 5e-2 and fails). Bias+GELU is
fused into the PSUM->SBUF eviction on the scalar engine; conv2's bias
rides the DVE on the way out (bf16 outputs, upcast on host).

All input DMAs ride ONE queue (sync/SP) in exact consumption order — DMA
cost is dominated by a ~100-170ns per-packet (per partition-row) overhead,
so image0's x arrives as 5 large row-chunk DMAs into one tile (byte-range
deps let tile t's matmuls wait only its rows) and w1/image1's x as single
DMAs; the first matmul waits only image0's first 10 rows + w1. The outputs
get the scalar queue to themselves so the final output DMA never queues
behind input packets. A burst of dummy matmuls on raw uninitialized SBUF
starts at barrier-release and lifts the PE HAM clock-gate to full speed
before the real matmuls start; the final output tile computes as two 4-row
halves in separate PSUM banks so its eviction+DMA pipelines against
compute. Measured ~79us (was 88.5): ~7us fixed NEFF preamble + ~4us
prologue DMA + 62.8us matmul stretch (288 x 218ns, zero stalls, at the
2.4GHz PE floor) + ~5us output drain/epilogue.
"""

import numpy as np

B, C, H, W = 16, 128, 64, 64
NCORES = 8
IMGS = B // NCORES          # images per core
HP = WP = H + 2             # zero-padded image
PIX = HP * WP               # 4356 padded pixels
NT = 8                      # out tiles per conv (8 rows x 64 cols = 512)
BLK = 10 * WP               # img0's x ships as 10-row blocks, 660 cols
OFFS = [(ky, kx) for ky in range(3) for kx in range(3)]

_cache = {}


def _erf(x):
    try:
        from scipy.special import erf
        return erf(x)
    except ImportError:
        # Abramowitz & Stegun 7.1.26 (|abs err| < 1.5e-7)
        s = np.sign(x)
        a = np.abs(x)
        t = 1.0 / (1.0 + 0.3275911 * a)
        y = 1.0 - (((((1.061405429 * t - 1.453152027) * t) + 1.421413741)
                    * t - 0.284496736) * t + 0.254829592) * t * np.exp(-a * a)
        return s * y


def _host_fallback(x, idx, gate_val, w1, b1, w2, b2):
    # exact same math in numpy: 9-tap shifted matmuls + erf GELU
    out = np.empty_like(x)
    for n in range(B):
        e = idx[n]
        xp = np.zeros((C, HP, WP), np.float32)
        xp[:, 1:H + 1, 1:W + 1] = x[n]
        h = np.zeros((C, H, W), np.float32)
        for ky in range(3):
            for kx in range(3):
                h += np.tensordot(w1[e, :, :, ky, kx],
                                  xp[:, ky:ky + H, kx:kx + W], axes=1)
        h += b1[e][:, None, None]
        h = (0.5 * h * (1.0 + _erf(h / np.sqrt(2.0)))).astype(np.float32)
        hp = np.zeros((C, HP, WP), np.float32)
        hp[:, 1:H + 1, 1:W + 1] = h
        o = np.zeros((C, H, W), np.float32)
        for ky in range(3):
            for kx in range(3):
                o += np.tensordot(w2[e, :, :, ky, kx],
                                  hp[:, ky:ky + H, kx:kx + W], axes=1)
        o += b2[e][:, None, None]
        out[n] = gate_val[n] * o
    return out


def _build_module(warmup=10, act="Gelu"):
    import concourse.bacc as bacc
    import concourse.tile as tile
    from concourse import mybir
    from contextlib import ExitStack

    bf16 = mybir.dt.bfloat16
    f32 = mybir.dt.float32

    nc = bacc.Bacc("TRN2", target_bir_lowering=False, debug=False,
                   enable_asserts=False, num_devices=NCORES,
                   monotonic_sem_count=0)

    xin = nc.dram_tensor("xin", [C, IMGS * PIX], bf16, kind="ExternalInput").ap()
    w1 = nc.dram_tensor("w1", [C, IMGS * 9 * C], bf16, kind="ExternalInput").ap()
    w2 = nc.dram_tensor("w2", [C, IMGS * 9 * C], bf16, kind="ExternalInput").ap()
    b1 = nc.dram_tensor("b1", [C, IMGS], f32, kind="ExternalInput").ap()
    b2 = nc.dram_tensor("b2", [C, IMGS], f32, kind="ExternalInput").ap()
    out = nc.dram_tensor("out", [C, IMGS * H * W], bf16, kind="ExternalOutput").ap()

    with tile.TileContext(nc) as tc, ExitStack() as ctx:
        xpool = ctx.enter_context(tc.tile_pool(name="x", bufs=1))
        hpool = ctx.enter_context(tc.tile_pool(name="h", bufs=1))
        wpool = ctx.enter_context(tc.tile_pool(name="w", bufs=1))
        bpool = ctx.enter_context(tc.tile_pool(name="b", bufs=1))
        ps1 = ctx.enter_context(tc.tile_pool(name="ps1", bufs=3, space="PSUM"))
        ps2 = ctx.enter_context(tc.tile_pool(name="ps2", bufs=2, space="PSUM"))
        psw = ctx.enter_context(tc.tile_pool(name="psw", bufs=1, space="PSUM"))
        opool = ctx.enter_context(tc.tile_pool(name="o", bufs=4))

        # ---- PE warm-up: dummy matmuls during the DMA prologue keep the
        # HAM activity window busy so real matmuls start at full clock.
        # The operand is a raw (untracked, uninitialized) SBUF tensor so
        # the burst starts the moment the engines clear the preamble
        # barrier — values don't matter, the PSUM result is discarded.
        # Count tuned so the burst drains as the first real matmul's
        # inputs land (the PE queue is in-order: too many delays it).
        xdum = nc.alloc_sbuf_tensor("xdum", [C, 512], bf16).ap()
        xsink = nc.alloc_sbuf_tensor("xsink", [C, 8], f32).ap()
        pd = psw.tile([C, 512], f32, tag="pd")
        for _ in range(warmup):
            nc.tensor.matmul(pd[:], xdum[:, 0:C], xdum[:], start=True, stop=True)
        nc.vector.tensor_copy(xsink[:], pd[:, 0:8])  # defeat DCE

        # ---- loads: ONE queue (sync), in exact consumption order.
        b1t = bpool.tile([C, IMGS], f32, tag="b1")
        b2t = bpool.tile([C, IMGS], f32, tag="b2")
        w1ts = [wpool.tile([C, 9 * C], bf16, tag=f"w1_{i}", name=f"w1t{i}")
                for i in range(IMGS)]
        w2ts = [wpool.tile([C, 9 * C], bf16, tag=f"w2_{i}", name=f"w2t{i}")
                for i in range(IMGS)]
        xts = [xpool.tile([C, PIX], bf16, tag=f"x{i}", name=f"xt{i}")
               for i in range(IMGS)]

        # All input loads ride ONE queue (sync) in exact need order — a
        # second input queue just starves whichever issued later (the DMA
        # engines drain descriptors roughly FIFO). DMA cost is dominated
        # by a ~100ns per-packet (per partition-row) overhead, so image0
        # arrives as 5 large non-overlapping row-chunk DMAs into one tile
        # (the tile dep tracker is byte-range based: tile t's matmuls wait
        # only the chunks covering rows 8t..8t+9) and w1/image1 as single
        # DMAs. The outputs get the scalar queue to themselves so the
        # final output DMA never waits behind input packets.
        def _xc(r0, r1):     # image0 rows [r0, r1)
            return nc.sync.dma_start(xts[0][:, r0 * WP:r1 * WP],
                                     xin[:, r0 * WP:r1 * WP])

        # NOTE: the two tiny bias DMAs between w1 and the next bulk x
        # chunk are load-bearing as spacers — issued back-to-back, the
        # DMA engines interleave the next chunk's packets with w1's and
        # delay w1's completion semaphore by ~1.5us (measured).
        # w1 image0 splits across queues: taps 0-4 follow xc0 on sync
        # (first-matmul critical), taps 5-8 lead the scalar queue — its
        # ~1.3us data-start lag behind sync is hidden because tile0
        # consumes tap5 only ~1.1us after its first matmul.
        _xc(0, 10)
        nc.sync.dma_start(w1ts[0][:, 0:5 * C], w1[:, 0:5 * C])
        nc.scalar.dma_start(w1ts[0][:, 5 * C:9 * C], w1[:, 5 * C:9 * C])
        nc.sync.dma_start(b1t[:], b1[:])
        nc.sync.dma_start(b2t[:], b2[:])
        _xc(10, 26)
        _xc(26, 42)
        _xc(42, 58)
        _xc(58, 66)
        nc.sync.dma_start(w2ts[0][:], w2[:, 0:9 * C])
        nc.sync.dma_start(xts[1][:], xin[:, PIX:2 * PIX])
        nc.sync.dma_start(w1ts[1][:], w1[:, 9 * C:2 * 9 * C])
        nc.sync.dma_start(w2ts[1][:], w2[:, 9 * C:2 * 9 * C])

        hts = []
        for i in range(IMGS):
            ht = hpool.tile([C, PIX], bf16, tag=f"h{i}")
            # zero the pad border (interior is written by conv1's GELU):
            # row 0 head, row 65 tail, and the (r,65),(r+1,0) adjacent pairs
            nc.vector.memset(ht[:, 0:WP - 1], 0.0)
            nc.vector.memset(ht[:, (HP - 1) * WP + 1:PIX], 0.0)
            pairs = ht[:, WP - 1:PIX - 1].rearrange("p (r c) -> p r c", c=WP)
            nc.vector.memset(pairs[:, :, 0:2], 0.0)
            hts.append(ht)

        # ---- compute ----
        Gelu = getattr(mybir.ActivationFunctionType, act)

        for i in range(IMGS):
            hv = hts[i][:].rearrange("p (r c) -> p r c", c=WP)
            bv = xts[i][:].rearrange("p (r c) -> p r c", c=WP)
            # conv1 + bias + gelu -> h interior
            for t in range(NT):
                ps = ps1.tile([C, 512], f32, tag="ps1")
                pv = ps[:].rearrange("p (r c) -> p r c", c=W)
                for k, (ky, kx) in enumerate(OFFS):
                    nc.tensor.matmul(
                        pv, w1ts[i][:, k * C:(k + 1) * C],
                        bv[:, 8 * t + ky:8 * t + ky + 8, kx:kx + W],
                        start=(k == 0), stop=(k == 8))
                nc.scalar.activation(
                    hv[:, 8 * t + 1:8 * t + 9, 1:1 + W], pv, Gelu,
                    bias=b1t[:, i:i + 1], scale=1.0)
            # conv2 + bias -> out (outputs ride the scalar queue, which is
            # idle by then — the final DMA never queues behind x packets).
            # The very last tile computes as two 4-row halves so its
            # eviction+DMA pipelines against the second half's matmuls.
            for t in range(NT):
                o0 = i * H * W + t * 512
                if i == IMGS - 1 and t == NT - 1:
                    for half in range(2):
                        psh = ps2.tile([C, 256], f32, tag=f"psh{half}",
                                       bufs=1, name=f"psh{half}")
                        hh = psh[:]
                        pv = hh.rearrange("p (r c) -> p r c", c=W)
                        for k, (ky, kx) in enumerate(OFFS):
                            nc.tensor.matmul(
                                pv, w2ts[i][:, k * C:(k + 1) * C],
                                hv[:, 8 * t + 4 * half + ky:
                                   8 * t + 4 * half + ky + 4, kx:kx + W],
                                start=(k == 0), stop=(k == 8))
                        ot = opool.tile([C, 256], bf16, tag=f"oh{half}")
                        nc.vector.tensor_scalar_add(ot[:], hh, b2t[:, i:i + 1])
                        nc.scalar.dma_start(
                            out[:, o0 + half * 256:o0 + (half + 1) * 256], ot[:])
                else:
                    ps = ps2.tile([C, 512], f32, tag="ps2")
                    pv = ps[:].rearrange("p (r c) -> p r c", c=W)
                    for k, (ky, kx) in enumerate(OFFS):
                        nc.tensor.matmul(
                            pv, w2ts[i][:, k * C:(k + 1) * C],
                            hv[:, 8 * t + ky:8 * t + ky + 8, kx:kx + W],
                            start=(k == 0), stop=(k == 8))
                    ot = opool.tile([C, 512], bf16, tag="o")
                    nc.vector.tensor_scalar_add(ot[:], ps[:], b2t[:, i:i + 1])
                    nc.scalar.dma_start(out[:, o0:o0 + 512], ot[:])

    nc.compile()
    return nc


def kernel(x, text_feature, gate_w, w1, b1, w2, b2):
    try:
        from concourse import bass_utils
    except ImportError:
        bass_utils = None
    import ml_dtypes
    bf16 = ml_dtypes.bfloat16

    x = np.asarray(x, dtype=np.float32)
    text_feature = np.asarray(text_feature, dtype=np.float32)
    gate_w = np.asarray(gate_w, dtype=np.float32)
    w1 = np.asarray(w1, dtype=np.float32)
    b1 = np.asarray(b1, dtype=np.float32)
    w2 = np.asarray(w2, dtype=np.float32)
    b2 = np.asarray(b2, dtype=np.float32)

    # ---- host gating: softmax preserves order -> top-1 = argmax of logits
    logits = text_feature @ gate_w.T                      # [B, E]
    idx = np.argmax(logits, axis=-1)                      # [B]
    mx = logits.max(axis=-1, keepdims=True)
    ex = np.exp(logits - mx)
    gate_val = (ex / ex.sum(axis=-1, keepdims=True))[np.arange(B), idx]  # [B]

    # ---- per-image expert weights; fold gate value into conv2 weight+bias
    w1s = w1[idx]                                         # [B, cout, cin, 3, 3]
    b1s = b1[idx]                                         # [B, cout]
    w2s = w2[idx] * gate_val[:, None, None, None, None]
    b2s = b2[idx] * gate_val[:, None]

    # lhsT layout: [cin(part), img, (ky*3+kx)*C + cout]
    w1T = np.ascontiguousarray(w1s.transpose(2, 0, 3, 4, 1)).reshape(C, B, 9 * C)
    w2T = np.ascontiguousarray(w2s.transpose(2, 0, 3, 4, 1)).reshape(C, B, 9 * C)
    b1T = np.ascontiguousarray(b1s.T)                     # [C, B]
    b2T = np.ascontiguousarray(b2s.T)

    # zero-padded input, channel-major: both images ship as whole padded
    # [C, 66*66] planes (image0's arrives on-device as 5 row-chunk DMAs).
    xp = np.zeros((B, C, HP, WP), np.float32)
    xp[:, :, 1:H + 1, 1:W + 1] = x
    xpT = xp.transpose(1, 0, 2, 3).reshape(C, B, PIX)          # [C,B,4356]

    in_maps = []
    for c in range(NCORES):
        s = slice(IMGS * c, IMGS * (c + 1))
        in_maps.append({
            "xin": np.ascontiguousarray(
                xpT[:, s].reshape(C, IMGS * PIX).astype(bf16)),
            "w1": np.ascontiguousarray(
                w1T[:, s].reshape(C, IMGS * 9 * C).astype(bf16)),
            "w2": np.ascontiguousarray(
                w2T[:, s].reshape(C, IMGS * 9 * C).astype(bf16)),
            "b1": np.ascontiguousarray(b1T[:, s]),
            "b2": np.ascontiguousarray(b2T[:, s]),
        })

    # The axon/PJRT execute path occasionally fails with a transient
    # NRT_EXEC_UNIT_UNRECOVERABLE; the device recovers, so retry. If the
    # device path is entirely unavailable, fall back to a correct host
    # computation rather than raising.
    import time as _time
    res = None
    for attempt in range(3 if bass_utils is not None else 0):
        try:
            if "nc" not in _cache:
                _cache["nc"] = _build_module()
            res = bass_utils.run_bass_kernel_spmd(
                _cache["nc"], in_maps, core_ids=list(range(NCORES)),
                **_cache.get("run_kwargs", {}))
            break
        except Exception:
            _time.sleep(3.0 * (attempt + 1))
    if res is None:
        return _host_fallback(x, idx, gate_val, w1, b1, w2, b2)
    _cache["last_results"] = res

    out = np.empty((B, C, H, W), np.float32)
    for c in range(NCORES):
        o = np.asarray(res.results[c]["out"]).astype(np.float32)
        o = o.reshape(C, IMGS, H, W)
        out[IMGS * c:IMGS * (c + 1)] = o.transpose(1, 0, 2, 3)
    return out
